# revision 8
# baseline (speedup 1.0000x reference)
"""Trainium2 Bass kernel for batched channel ("XCA"-style) attention.

Reference computation (per batch b; B=8, A=2048 tokens, D=1024 dims):
    q = x @ Wq.T ; k = x @ Wk.T ; v = x @ Wv.T          # (A, D)
    q,k,v -> (D, A); q,k L2-normalized over the token axis
    attn = softmax((qn @ kn.T) * temperature, axis=-1)   # (D, D)
    out  = attn @ v_da                                   # (D, A)
    y    = out.T @ Wo.T                                  # (A, D)

Sharding: pure data parallelism -- batch b -> NeuronCore b (8 cores, no
collectives). Host pre-transposes x and the weights so all device DMAs are
contiguous; all matmuls run in bf16 (fp32 PSUM accumulation), softmax and
norm math in fp32.

Device-side layout plan (per core):
  xT   (e, a) = x[b].T            : stationary for q/k projections, moving for v
  q_ad (a, d), k_ad (a, d)        : lhsT/rhs of the scores matmul (contract a)
  sumsq over tokens via ACT Square + ones-vector matmul (column reduction)
  S (d, d') in PSUM -> DVE mul by bcast(1/nk[d']) -> ACT Exp with
      scale = temperature/nq[d] (per-partition) and fused accum_out = denom
  P -> PE transpose -> PT (d', d)
  v_da (d, a) = WvT.T @ xT
  out_da = PT.T @ v_da, scaled by 1/denom[d] on eviction
  y (a, f) = out_da.T @ WoT, DMA'd out as the (A, D) result
"""

import numpy as np

B, A, D = 8, 2048, 1024
P = 128
E_T = D // P     # 8 tiles along the contraction (feature) dim
A_T = A // P     # 16 tiles along the token dim
D_T = D // P     # 8 tiles along the channel dim
NCH = 512        # matmul moving-operand chunk (one PSUM bank of fp32)

_CACHE = {}


def _ensure_path():
    import sys
    for p in ("/opt/trn_rl_repo",):
        if p not in sys.path:
            sys.path.insert(0, p)


def build_bass():
    """Build the single-core Bass/Tile graph (SPMD across 8 cores)."""
    _ensure_path()
    import concourse.bacc as bacc
    import concourse.mybir as mybir
    import concourse.tile as tile
    from concourse.masks import make_identity

    dt = mybir.dt
    BF = dt.bfloat16
    F32 = dt.float32
    AF = mybir.ActivationFunctionType
    MULT = mybir.AluOpType.mult

    nc = bacc.Bacc()

    xT_d = nc.declare_dram_parameter("xT", [D, A], BF, isOutput=False)
    wq_d = nc.declare_dram_parameter("wqT", [D, D], BF, isOutput=False)
    wk_d = nc.declare_dram_parameter("wkT", [D, D], BF, isOutput=False)
    wv_d = nc.declare_dram_parameter("wvT", [D, D], BF, isOutput=False)
    wo_d = nc.declare_dram_parameter("woT", [D, D], BF, isOutput=False)
    tp_d = nc.declare_dram_parameter("temp", [1, 1], F32, isOutput=False)
    out_d = nc.declare_dram_parameter("out", [A, D], F32, isOutput=True)

    with tile.TileContext(nc) as tc:
        # ---- long-lived pools (opened bottom of the stack) ----
        consts = tc.alloc_tile_pool(name="consts", bufs=1)
        pt_pool = tc.alloc_tile_pool(name="pt", bufs=1)
        # xT lives on the opposite SBUF side: its release (end of phase 3) is
        # non-LIFO w.r.t. the left-side pool stack.
        xT_pool = tc.alloc_tile_pool(name="xTp", bufs=1, side="right")
        w_pool = tc.alloc_tile_pool(name="w", bufs=1)
        q_pool = tc.alloc_tile_pool(name="qp", bufs=1)
        k_pool = tc.alloc_tile_pool(name="kp", bufs=1)
        misc = tc.alloc_tile_pool(name="misc", bufs=2)

        # constants
        ident = consts.tile([P, P], BF, tag="ident")
        make_identity(nc, ident)
        ones_col = consts.tile([P, 1], BF, tag="ones_col")
        nc.vector.memset(ones_col[:], 1.0)
        ones_row = consts.tile([1, P], F32, tag="ones_row")
        nc.vector.memset(ones_row[:], 1.0)
        one11 = consts.tile([1, 1], F32, tag="one11")
        nc.vector.memset(one11[:], 1.0)
        t_sb = consts.tile([1, 1], F32, tag="t_sb")
        nc.sync.dma_start(t_sb[:], tp_d[:])
        denom = consts.tile([P, D_T], F32, tag="denom")
        invden = consts.tile([P, D_T], F32, tag="invden")
        invnq_col = consts.tile([P, D_T], F32, tag="invnq_col")

        # resident tensors
        xT_sb = xT_pool.tile([P, E_T, A], BF, tag="xT")
        for e in range(E_T):
            nc.sync.dma_start(xT_sb[:, e, :], xT_d[e * P:(e + 1) * P, :])

        q_sb = q_pool.tile([P, A_T, D], BF, tag="q")
        k_sb = k_pool.tile([P, A_T, D], BF, tag="k")

        # ---------- phase 1: q/k projections + token-axis sumsq ----------
        qk_ps = tc.alloc_tile_pool(name="qk_ps", bufs=2, space="PSUM")
        nrm_ps = tc.alloc_tile_pool(name="nrm_ps", bufs=1, space="PSUM")
        sq_pool = tc.alloc_tile_pool(name="sq", bufs=1)

        def proj_pass(w_dram, dst_sb, inv_row_out):
            """One projection pass (q or k): MMs, evictions, squares, norm."""
            w_sb = w_pool.tile([P, E_T, D], BF, tag="w")
            for e in range(E_T):
                nc.sync.dma_start(w_sb[:, e, :], w_dram[e * P:(e + 1) * P, :])
            sq_sb = sq_pool.tile([P, A_T, D], BF, tag="sq")
            for ai in range(A_T):
                acc = qk_ps.tile([P, D], mybir.dt.float32, tag="qk")
                for e in range(E_T):
                    lhs = xT_sb[:, e, ai * P:(ai + 1) * P]
                    for c in range(D // NCH):
                        nc.tensor.matmul(
                            acc[:, c * NCH:(c + 1) * NCH],
                            lhs,
                            w_sb[:, e, c * NCH:(c + 1) * NCH],
                            start=(e == 0),
                            stop=(e == E_T - 1),
                        )
                nc.vector.tensor_copy(dst_sb[:, ai, :], acc[:])
                nc.scalar.activation(sq_sb[:, ai, :], acc[:], AF.Square)
            # token-axis (partition) reduction of squares via ones-matmul
            ns = nrm_ps.tile([1, D], mybir.dt.float32, tag="nsum")
            for ai in range(A_T):
                for c in range(D // NCH):
                    nc.tensor.matmul(
                        ns[:, c * NCH:(c + 1) * NCH],
                        ones_col[:],
                        sq_sb[:, ai, c * NCH:(c + 1) * NCH],
                        start=(ai == 0),
                        stop=(ai == A_T - 1),
                    )
            # 1/sqrt(sumsq)
            n_row = misc.tile([1, D], mybir.dt.float32, tag="row")
            nc.scalar.activation(n_row[:], ns[:], AF.Sqrt)
            nc.vector.reciprocal(inv_row_out[:], n_row[:])

        invnq_row = misc.tile([1, D], F32, tag="invrow")
        proj_pass(wq_d, q_sb, invnq_row)
        # fold temperature into the q-side scale
        nc.vector.tensor_scalar(
            out=invnq_row[:], in0=invnq_row[:],
            scalar1=t_sb[0:1, 0:1], scalar2=None, op0=MULT,
        )
        # column-ize 1/nq for use as per-partition ACT scale: 8 tiny transposes
        icol_ps = nrm_ps.tile([P, D_T], F32, tag="invcol")
        for j in range(D_T):
            nc.tensor.transpose(
                icol_ps[:, j:j + 1],
                invnq_row[0:1, j * P:(j + 1) * P],
                one11[:],
            )
        nc.vector.tensor_copy(invnq_col[:], icol_ps[:])

        invnk_row = misc.tile([1, D], F32, tag="invrow")
        proj_pass(wk_d, k_sb, invnk_row)

        # broadcast 1/nk along partitions via K=1 matmul -> (P, D) fp32
        bc_ps = qk_ps.tile([P, D], F32, tag="qk")
        for c in range(D // NCH):
            nc.tensor.matmul(
                bc_ps[:, c * NCH:(c + 1) * NCH],
                ones_row[:],
                invnk_row[0:1, c * NCH:(c + 1) * NCH],
            )

        sq_pool.release()
        s_pool = tc.alloc_tile_pool(name="s_scr", bufs=2)
        bcast_sb = s_pool.tile([P, D], F32, tag="bcast")
        nc.vector.tensor_copy(bcast_sb[:], bc_ps[:])

        nrm_ps.release()
        qk_ps.release()

        # ---------- phase 2: scores + softmax ----------
        p_pool = tc.alloc_tile_pool(name="pp", bufs=1)
        p_sb = p_pool.tile([P, D_T, D], BF, tag="p")

        s_ps_pool = tc.alloc_tile_pool(name="s_ps", bufs=2, space="PSUM")
        pt_ps_pool = tc.alloc_tile_pool(name="pt_ps", bufs=2, space="PSUM")

        for dj in range(D_T):
            s_ps = s_ps_pool.tile([P, D], F32, tag="s")
            for ai in range(A_T):
                lhs = q_sb[:, ai, dj * P:(dj + 1) * P]
                for c in range(D // NCH):
                    nc.tensor.matmul(
                        s_ps[:, c * NCH:(c + 1) * NCH],
                        lhs,
                        k_sb[:, ai, c * NCH:(c + 1) * NCH],
                        start=(ai == 0),
                        stop=(ai == A_T - 1),
                    )
            # S * (1/nk[d']) with partition-broadcast tile, then
            # P = exp(S * temp/nq[d]) with fused row-sum (softmax denominator)
            s_scr = s_pool.tile([P, D], F32, tag="s_scr")
            nc.vector.tensor_tensor(s_scr[:], s_ps[:], bcast_sb[:], MULT)
            nc.scalar.activation(
                p_sb[:, dj, :], s_scr[:], AF.Exp,
                scale=invnq_col[:, dj:dj + 1],
                accum_out=denom[:, dj:dj + 1],
            )
        nc.vector.reciprocal(invden[:], denom[:])

        # transpose P -> PT (d', d)
        pt_sb = pt_pool.tile([P, D_T, D], BF, tag="pt")
        for di in range(D_T):
            tp = pt_ps_pool.tile([P, D], BF, tag="ptp")
            for dj in range(D_T):
                nc.tensor.transpose(
                    tp[:, dj * P:(dj + 1) * P],
                    p_sb[:, dj, di * P:(di + 1) * P],
                    ident[:],
                )
            nc.vector.tensor_copy(pt_sb[:, di, :], tp[:])

        pt_ps_pool.release()
        s_ps_pool.release()
        p_pool.release()
        s_pool.release()
        misc.release()
        k_pool.release()
        q_pool.release()

        # ---------- phase 3: v projection (d, a layout) ----------
        v_pool = tc.alloc_tile_pool(name="vp", bufs=1)
        v_sb = v_pool.tile([P, D_T, A], BF, tag="v")
        v_ps_pool = tc.alloc_tile_pool(name="v_ps", bufs=2, space="PSUM")
        wv_sb = w_pool.tile([P, E_T, D], BF, tag="w")
        for e in range(E_T):
            nc.sync.dma_start(wv_sb[:, e, :], wv_d[e * P:(e + 1) * P, :])
        for dj in range(D_T):
            vp = v_ps_pool.tile([P, A], F32, tag="vps")
            for e in range(E_T):
                lhs = wv_sb[:, e, dj * P:(dj + 1) * P]
                for c in range(A // NCH):
                    nc.tensor.matmul(
                        vp[:, c * NCH:(c + 1) * NCH],
                        lhs,
                        xT_sb[:, e, c * NCH:(c + 1) * NCH],
                        start=(e == 0),
                        stop=(e == E_T - 1),
                    )
            nc.vector.tensor_copy(v_sb[:, dj, :], vp[:])
        v_ps_pool.release()
        xT_pool.release()

        # ---------- phase 4: out_da = P @ v_da (via PT), / denom ----------
        o_pool = tc.alloc_tile_pool(name="op", bufs=1)
        o_sb = o_pool.tile([P, D_T, A], BF, tag="o")
        o_ps_pool = tc.alloc_tile_pool(name="o_ps", bufs=2, space="PSUM")
        for dj in range(D_T):
            op = o_ps_pool.tile([P, A], F32, tag="ops")
            for di in range(D_T):
                lhs = pt_sb[:, di, dj * P:(dj + 1) * P]
                for c in range(A // NCH):
                    nc.tensor.matmul(
                        op[:, c * NCH:(c + 1) * NCH],
                        lhs,
                        v_sb[:, di, c * NCH:(c + 1) * NCH],
                        start=(di == 0),
                        stop=(di == D_T - 1),
                    )
            nc.vector.tensor_scalar(
                out=o_sb[:, dj, :], in0=op[:],
                scalar1=invden[:, dj:dj + 1], scalar2=None, op0=MULT,
            )
        o_ps_pool.release()

        # ---------- phase 5: y = out_ad @ Wo.T ----------
        wo_sb = w_pool.tile([P, E_T, D], BF, tag="w")
        for e in range(E_T):
            nc.sync.dma_start(wo_sb[:, e, :], wo_d[e * P:(e + 1) * P, :])
        y_pool = tc.alloc_tile_pool(name="yp", bufs=2)
        y_ps_pool = tc.alloc_tile_pool(name="y_ps", bufs=2, space="PSUM")
        for ai in range(A_T):
            yp = y_ps_pool.tile([P, D], F32, tag="yps")
            for dj in range(D_T):
                lhs = o_sb[:, dj, ai * P:(ai + 1) * P]
                for c in range(D // NCH):
                    nc.tensor.matmul(
                        yp[:, c * NCH:(c + 1) * NCH],
                        lhs,
                        wo_sb[:, dj, c * NCH:(c + 1) * NCH],
                        start=(dj == 0),
                        stop=(dj == D_T - 1),
                    )
            y_sb = y_pool.tile([P, D], F32, tag="y")
            nc.vector.tensor_copy(y_sb[:], yp[:])
            nc.sync.dma_start(out_d[ai * P:(ai + 1) * P, :], y_sb[:])
        y_ps_pool.release()
        y_pool.release()
        o_pool.release()
        v_pool.release()
        w_pool.release()
        pt_pool.release()
        consts.release()

    nc.compile()
    return nc


def _host_inputs(x, Wq, Wk, Wv, Wo, temperature):
    import ml_dtypes
    bf16 = ml_dtypes.bfloat16
    wqT = np.ascontiguousarray(np.asarray(Wq).T).astype(bf16)
    wkT = np.ascontiguousarray(np.asarray(Wk).T).astype(bf16)
    wvT = np.ascontiguousarray(np.asarray(Wv).T).astype(bf16)
    woT = np.ascontiguousarray(np.asarray(Wo).T).astype(bf16)
    in_maps = []
    for b in range(B):
        in_maps.append({
            "xT": np.ascontiguousarray(np.asarray(x[b]).T).astype(bf16),
            "wqT": wqT,
            "wkT": wkT,
            "wvT": wvT,
            "woT": woT,
            "temp": np.asarray(temperature[b]).reshape(1, 1).astype(np.float32),
        })
    return in_maps


def run(x, Wq, Wk, Wv, Wo, temperature, trace=False, tmpdir=None):
    """Run on the 8 NeuronCores; returns (out, BassKernelResults)."""
    _ensure_path()
    from concourse.bass_utils import run_bass_kernel_spmd

    if "nc" not in _CACHE:
        _CACHE["nc"] = build_bass()
    nc = _CACHE["nc"]
    in_maps = _host_inputs(x, Wq, Wk, Wv, Wo, temperature)
    res = run_bass_kernel_spmd(
        nc, in_maps, core_ids=list(range(B)), trace=trace, tmpdir=tmpdir
    )
    out = np.stack([np.asarray(res.results[b]["out"]) for b in range(B)])
    return out.astype(np.float32), res


def kernel(x, Wq, Wk, Wv, Wo, temperature):
    out, _ = run(x, Wq, Wk, Wv, Wo, temperature, trace=False)
    return out
